# revision 1
# baseline (speedup 1.0000x reference)
"""Trainium2 Bass kernel for nn_AutoRegerting_2954937500106.

Self-contained: builds an 8-core SPMD Bass program (tensor-parallel GRU
recurrence with per-step AllGather + V-sharded vocab head), runs it via
run_bass_kernel_spmd, and reassembles the full [B, T, V] output.
"""
import sys as _sys
for _p in ("/opt/trn_rl_repo", "/opt/trn_rl_repo/concourse"):
    if _p not in _sys.path:
        _sys.path.append(_p)

"""Bass/Tile TRN2 kernel for the 2-layer GRU + LN + vocab-head problem.

Strategy:
  - Recurrence: 8-way tensor-parallel over the gate/hidden dim. Each core
    computes a 384-row slice (128 rows of each of r/z/n) of both GRU layers'
    gate pre-activations; hidden-state chunks are exchanged every step with
    one AllGather carrying both layers' chunks; LayerNorm is computed
    redundantly on every core after the gather.
  - gi0 (input-side gates of layer 0) is precomputed batched over all T.
  - Head (Linear->LeakyReLU->LN->Linear(V)) runs batched over B*T after the
    recurrence, with the vocab projection sharded over V (4000 cols/core).
  - Layout: hidden state kept transposed ([H partitions, batch cols]) so
    gates/LN work on [128, 16] tiles and matmuls use weight-stationary form.

Host side: embedding gather, weight transposes/slicing, final concat +
transpose + b2 add.
"""
import numpy as np
import concourse.bacc as bacc
import concourse.bass as bass
import concourse.mybir as mybir
import concourse.tile as tile

F32 = mybir.dt.float32
AF = mybir.ActivationFunctionType
ALU = mybir.AluOpType

H = 1024
E = 512
B = 16
V = 32000
NCORES = 8
KH = H // 128    # 8 h-chunks
KE = E // 128    # 4 e-chunks
MSL = 3 * 128    # 384: per-core slice of the 3H gate dim
VC = V // NCORES # 4000
EPS = 1e-5
NEG_SLOPE = 0.01


def build_nc(T=256, n_cores=NCORES, rec_bf16=False):
    WDT = mybir.dt.float16 if rec_bf16 else F32
    BT = T * B
    nc = bacc.Bacc("TRN2", target_bir_lowering=False, debug=False,
                   enable_asserts=False, num_devices=n_cores)

    xT    = nc.dram_tensor("xT",    [KE, 128, BT], F32, kind="ExternalInput").ap()
    wih0  = nc.dram_tensor("wih0",  [E, MSL], F32, kind="ExternalInput").ap()
    whh0  = nc.dram_tensor("whh0",  [H, MSL], F32, kind="ExternalInput").ap()
    wih1  = nc.dram_tensor("wih1",  [H, MSL], F32, kind="ExternalInput").ap()
    whh1  = nc.dram_tensor("whh1",  [H, MSL], F32, kind="ExternalInput").ap()
    bih0c = nc.dram_tensor("bih0c", [128, 3], F32, kind="ExternalInput").ap()
    gb0   = nc.dram_tensor("gb0",   [128, 3], F32, kind="ExternalInput").ap()
    gb1   = nc.dram_tensor("gb1",   [128, 4], F32, kind="ExternalInput").ap()
    lnw   = nc.dram_tensor("lnw",   [128, 2, KH], F32, kind="ExternalInput").ap()
    lnb   = nc.dram_tensor("lnb",   [128, 2, KH], F32, kind="ExternalInput").ap()
    maskc = nc.dram_tensor("maskc", [128, KH], F32, kind="ExternalInput").ap()
    eye16 = nc.dram_tensor("eye16", [16, 16], F32, kind="ExternalInput").ap()
    ln2w  = nc.dram_tensor("ln2w",  [128, KH], F32, kind="ExternalInput").ap()
    ln2b  = nc.dram_tensor("ln2b",  [128, KH], F32, kind="ExternalInput").ap()
    b1c   = nc.dram_tensor("b1c",   [128, KH], F32, kind="ExternalInput").ap()
    w1T   = nc.dram_tensor("w1T",   [H, H], F32, kind="ExternalInput").ap()
    w2cT  = nc.dram_tensor("w2cT",  [H, VC], F32, kind="ExternalInput").ap()
    out   = nc.dram_tensor("out",   [BT, VC], F32, kind="ExternalOutput").ap()

    rg = [list(range(n_cores))]

    with tile.TileContext(nc) as tc:
        # ---- persistent DRAM scratch ----
        with tc.tile_pool(name="dramp", bufs=1, space="DRAM") as dramp:
            gi0T = dramp.tile([3, 128, BT], F32)
            h1T  = dramp.tile([KH, 128, BT], F32)
            aTn  = dramp.tile([KH, 128, BT], F32)

            # ================= Phase 0: batched gi0 =================
            with tc.tile_pool(name="p0", bufs=1) as p0, \
                 tc.tile_pool(name="p0w", bufs=2) as p0w, \
                 tc.tile_pool(name="ps0", bufs=2, space="PSUM") as ps0:
                xs = p0.tile([128, KE, BT], F32)
                nc.sync.dma_start(xs[:], xT.rearrange("k p n -> p k n"))
                wih0s = p0.tile([128, KE, MSL], F32)
                nc.sync.dma_start(wih0s[:], wih0.rearrange("(k p) m -> p k m", p=128))
                bih0s = p0.tile([128, 3], F32)
                nc.sync.dma_start(bih0s[:], bih0c[:])
                nbt = (BT + 511) // 512
                assert BT % nbt == 0
                btc = BT // nbt
                for m in range(3):
                    for j in range(nbt):
                        ps = ps0.tile([128, btc], F32, tag="ps")
                        for k in range(KE):
                            nc.tensor.matmul(
                                ps[:],
                                wih0s[:, k, m * 128:(m + 1) * 128],
                                xs[:, k, j * btc:(j + 1) * btc],
                                start=(k == 0), stop=(k == KE - 1))
                        gtile = p0w.tile([128, btc], F32, tag="gt")
                        nc.scalar.activation(gtile[:], ps[:], AF.Identity,
                                             bias=bih0s[:, m:m + 1])
                        nc.sync.dma_start(gi0T[m, :, j * btc:(j + 1) * btc], gtile[:])

            # ================= Phase 1: recurrence =================
            with tc.tile_pool(name="rw", bufs=1) as rw, \
                 tc.tile_pool(name="rs", bufs=2) as rs, \
                 tc.tile_pool(name="rtmp", bufs=3) as rtmp, \
                 tc.tile_pool(name="rps", bufs=1, space="PSUM") as rps, \
                 tc.tile_pool(name="rpst", bufs=1, space="PSUM") as rpst, \
                 tc.tile_pool(name="rdram", bufs=3, space="DRAM") as rdram:

                whh0s = rw.tile([128, KH, MSL], WDT)
                wih1s = rw.tile([128, KH, MSL], WDT)
                whh1s = rw.tile([128, KH, MSL], WDT)
                with tc.tile_pool(name="wstg", bufs=2) as wstg:
                    for wsrc, wdst in ((whh0, whh0s), (wih1, wih1s),
                                       (whh1, whh1s)):
                        wtmp = wstg.tile([128, KH, MSL], F32, tag="wtmp",
                                         name="wtmp")
                        nc.sync.dma_start(
                            wtmp[:], wsrc.rearrange("(k p) m -> p k m", p=128))
                        nc.vector.tensor_copy(wdst[:], wtmp[:])
                gb0s = rw.tile([128, 3], F32)
                nc.sync.dma_start(gb0s[:], gb0[:])
                gb1s = rw.tile([128, 4], F32)
                nc.sync.dma_start(gb1s[:], gb1[:])
                lnws = rw.tile([128, 2, KH], F32)
                nc.sync.dma_start(lnws[:], lnw[:])
                lnbs = rw.tile([128, 2, KH], F32)
                nc.sync.dma_start(lnbs[:], lnb[:])
                masks = rw.tile([128, KH], F32)
                nc.sync.dma_start(masks[:], maskc[:])
                eyes = rw.tile([16, 16], F32)
                nc.sync.dma_start(eyes[:], eye16[:])
                ones_col = rw.tile([128, 1], F32)
                nc.vector.memset(ones_col[:], 1.0)
                ones_row = rw.tile([1, 128], F32)
                nc.vector.memset(ones_row[:], 1.0)

                for k in range(T + 2):
                    last = (k == T + 1)
                    # ---- LN of AG#(k-1) -> h0[k-1], h1[k-2] ----
                    hn = rs.tile([128, 2, KH, B], F32, tag="hn")
                    if k == 0:
                        nc.vector.memset(hn[:], 0.0)
                    else:
                        hp = rtmp.tile([128, 2, KH, B], F32, tag="hp")
                        ag4 = agout[:].rearrange("(c l p) b -> p l c b", p=128, l=2)
                        nc.sync.dma_start(hp[:, 0], ag4[:, 0])
                        nc.sync.dma_start(hp[:, 1], ag4[:, 1])
                        sq = rtmp.tile([128, 2, KH, B], F32, tag="sq")
                        nc.scalar.activation(sq[:], hp[:], AF.Square)
                        ps_s = rpst.tile([1, 2, B, KH], F32, tag="pstat")
                        nc.tensor.matmul(ps_s[:], ones_col[:],
                                         hp.rearrange("p l c b -> p l b c"),
                                         start=True, stop=True)
                        ps_q = rpst.tile([1, 2, B, KH], F32, tag="pstat")
                        nc.tensor.matmul(ps_q[:], ones_col[:],
                                         sq.rearrange("p l c b -> p l b c"),
                                         start=True, stop=True)
                        mean = rtmp.tile([1, 2, B], F32, tag="mean")
                        nc.vector.tensor_reduce(mean[:], ps_s[:],
                                                mybir.AxisListType.X, ALU.add)
                        nc.vector.tensor_scalar_mul(mean[:], mean[:], 1.0 / H)
                        msq = rtmp.tile([1, 2, B], F32, tag="msq")
                        nc.vector.tensor_reduce(msq[:], ps_q[:],
                                                mybir.AxisListType.X, ALU.add)
                        nc.vector.tensor_scalar_mul(msq[:], msq[:], 1.0 / H)
                        var = rtmp.tile([1, 2, B], F32, tag="var")
                        rstd = rtmp.tile([1, 2, B], F32, tag="rstd")
                        nc.vector.tensor_mul(var[:], mean[:], mean[:])
                        nc.vector.tensor_sub(var[:], msq[:], var[:])
                        nc.vector.tensor_scalar_add(var[:], var[:], EPS)
                        nc.vector.reciprocal(var[:], var[:])
                        nc.scalar.activation(rstd[:], var[:], AF.Sqrt)
                        ps_mb = rpst.tile([128, 2, KH, B], F32, tag="pstat")
                        nc.tensor.matmul(
                            ps_mb[:], ones_row[:],
                            mean.unsqueeze(2).broadcast_to([1, 2, KH, B]),
                            start=True, stop=True)
                        ps_rb = rpst.tile([128, 2, KH, B], F32, tag="pstat")
                        nc.tensor.matmul(
                            ps_rb[:], ones_row[:],
                            rstd.unsqueeze(2).broadcast_to([1, 2, KH, B]),
                            start=True, stop=True)
                        nc.vector.tensor_sub(hn[:], hp[:], ps_mb[:])
                        nc.vector.tensor_mul(hn[:], hn[:], ps_rb[:])
                        nc.vector.tensor_mul(
                            hn[:], hn[:],
                            lnws.unsqueeze(3).broadcast_to([128, 2, KH, B]))
                        nc.vector.tensor_add(
                            hn[:], hn[:],
                            lnbs.unsqueeze(3).broadcast_to([128, 2, KH, B]))
                        if k == 1:
                            nc.vector.memset(hn[:, 1], 0.0)
                    hnb = rs.tile([128, 2, KH, B], WDT, tag="hnb")
                    nc.vector.tensor_copy(hnb[:], hn[:])
                    # store h1[k-2]
                    if 2 <= k <= T + 1:
                        nc.sync.dma_start(
                            h1T[:, :, (k - 2) * B:(k - 1) * B].rearrange(
                                "c p b -> p c b"),
                            hn[:, 1])
                    if last:
                        break

                    # ---- gate matmuls (h-stationary, stream weights) ----
                    pA0 = rps.tile([16, MSL], F32, tag="pA0")
                    pA1i = rps.tile([16, MSL], F32, tag="pA1i")
                    pA1h = rps.tile([16, MSL], F32, tag="pA1h")
                    h0a = hn[:, 0]
                    h1a = hn[:, 1]
                    for kk in range(KH):
                        nc.tensor.matmul(pA0[:], h0a[:, kk, :], whh0s[:, kk, :],
                                         start=(kk == 0), stop=(kk == KH - 1))
                    for kk in range(KH):
                        nc.tensor.matmul(pA1i[:], h0a[:, kk, :], wih1s[:, kk, :],
                                         start=(kk == 0), stop=(kk == KH - 1))
                    for kk in range(KH):
                        nc.tensor.matmul(pA1h[:], h1a[:, kk, :], whh1s[:, kk, :],
                                         start=(kk == 0), stop=(kk == KH - 1))
                    sA0 = rtmp.tile([16, MSL], F32, tag="sA0")
                    nc.vector.tensor_copy(sA0[:], pA0[:])
                    sA1i = rtmp.tile([16, MSL], F32, tag="sA1i")
                    nc.vector.tensor_copy(sA1i[:], pA1i[:])
                    sA1h = rtmp.tile([16, MSL], F32, tag="sA1h")
                    nc.vector.tensor_copy(sA1h[:], pA1h[:])
                    sA1rz = rtmp.tile([16, 256], F32, tag="sA1rz")
                    nc.vector.tensor_add(sA1rz[:], sA1i[:, 0:256], sA1h[:, 0:256])
                    # transpose to [128,16] gate tiles
                    pTa = rps.tile([128, 2, B], F32, tag="pTa")
                    pTb = rps.tile([128, 2, B], F32, tag="pTb")
                    pTc = rps.tile([128, 2, B], F32, tag="pTc")
                    pTd = rps.tile([128, B], F32, tag="pTd")
                    nc.tensor.transpose(pTa[:, 0], sA0[:, 0:128], eyes[:])
                    nc.tensor.transpose(pTa[:, 1], sA0[:, 128:256], eyes[:])
                    nc.tensor.transpose(pTb[:, 0], sA0[:, 256:384], eyes[:])
                    nc.tensor.transpose(pTb[:, 1], sA1i[:, 256:384], eyes[:])
                    nc.tensor.transpose(pTc[:, 0], sA1rz[:, 0:128], eyes[:])
                    nc.tensor.transpose(pTc[:, 1], sA1rz[:, 128:256], eyes[:])
                    nc.tensor.transpose(pTd[:], sA1h[:, 256:384], eyes[:])
                    bank_a = pTa
                    bank_c = pTc
                    bank_d = pTd

                    # ---- gi0 slice for this tick ----
                    tgi = min(k, T - 1)
                    gi0t = rtmp.tile([128, 3, B], F32, tag="gi0t")
                    nc.sync.dma_start(
                        gi0t[:], gi0T[:, :, tgi * B:(tgi + 1) * B].rearrange(
                            "m p b -> p m b"))

                    # ---- layer0 gates ----
                    t0r = rtmp.tile([128, B], F32, tag="t0r")
                    nc.vector.scalar_tensor_tensor(t0r[:], bank_a[:, 0],
                                                   gb0s[:, 0:1], gi0t[:, 0],
                                                   ALU.add, ALU.add)
                    r0 = rtmp.tile([128, B], F32, tag="r0")
                    nc.scalar.activation(r0[:], t0r[:], AF.Sigmoid)
                    t0z = rtmp.tile([128, B], F32, tag="t0z")
                    nc.vector.scalar_tensor_tensor(t0z[:], bank_a[:, 1],
                                                   gb0s[:, 1:2], gi0t[:, 1],
                                                   ALU.add, ALU.add)
                    z0 = rtmp.tile([128, B], F32, tag="z0")
                    nc.scalar.activation(z0[:], t0z[:], AF.Sigmoid)
                    hn0m = rtmp.tile([128, B], F32, tag="hn0m")
                    nc.vector.tensor_scalar_add(hn0m[:], pTb[:, 0], gb0s[:, 2:3])
                    nc.vector.tensor_mul(hn0m[:], hn0m[:], r0[:])
                    nc.vector.tensor_add(hn0m[:], hn0m[:], gi0t[:, 2])
                    n0 = rtmp.tile([128, B], F32, tag="n0")
                    nc.scalar.activation(n0[:], hn0m[:], AF.Tanh)
                    # h chunk select (this core's rows of h0[k-1])
                    hsel0 = rtmp.tile([128, KH, B], F32, tag="hsel0")
                    nc.vector.tensor_mul(
                        hsel0[:], hn[:, 0],
                        masks.unsqueeze(2).broadcast_to([128, KH, B]))
                    hc0 = rtmp.tile([128, B], F32, tag="hc0")
                    nc.vector.tensor_reduce(
                        hc0[:], hsel0.rearrange("p c b -> p b c"),
                        mybir.AxisListType.X, ALU.add)
                    h0p = rtmp.tile([128, B], F32, tag="h0p")
                    nc.vector.tensor_sub(h0p[:], hc0[:], n0[:])
                    nc.vector.tensor_mul(h0p[:], h0p[:], z0[:])
                    nc.vector.tensor_add(h0p[:], h0p[:], n0[:])

                    # ---- layer1 gates ----
                    r1 = rtmp.tile([128, B], F32, tag="r1")
                    nc.scalar.activation(r1[:], bank_c[:, 0], AF.Sigmoid,
                                         bias=gb1s[:, 0:1])
                    z1 = rtmp.tile([128, B], F32, tag="z1")
                    nc.scalar.activation(z1[:], bank_c[:, 1], AF.Sigmoid,
                                         bias=gb1s[:, 1:2])
                    hn1m = rtmp.tile([128, B], F32, tag="hn1m")
                    nc.vector.tensor_scalar_add(hn1m[:], bank_d[:], gb1s[:, 3:4])
                    nc.vector.tensor_mul(hn1m[:], hn1m[:], r1[:])
                    nc.vector.scalar_tensor_tensor(hn1m[:], pTb[:, 1],
                                                   gb1s[:, 2:3], hn1m[:],
                                                   ALU.add, ALU.add)
                    n1 = rtmp.tile([128, B], F32, tag="n1")
                    nc.scalar.activation(n1[:], hn1m[:], AF.Tanh)
                    hsel1 = rtmp.tile([128, KH, B], F32, tag="hsel1")
                    nc.vector.tensor_mul(
                        hsel1[:], hn[:, 1],
                        masks.unsqueeze(2).broadcast_to([128, KH, B]))
                    hc1 = rtmp.tile([128, B], F32, tag="hc1")
                    nc.vector.tensor_reduce(
                        hc1[:], hsel1.rearrange("p c b -> p b c"),
                        mybir.AxisListType.X, ALU.add)
                    h1p = rtmp.tile([128, B], F32, tag="h1p")
                    nc.vector.tensor_sub(h1p[:], hc1[:], n1[:])
                    nc.vector.tensor_mul(h1p[:], h1p[:], z1[:])
                    nc.vector.tensor_add(h1p[:], h1p[:], n1[:])

                    # ---- AllGather ----
                    agin = rdram.tile([2, 128, B], F32, tag="agin")
                    nc.sync.dma_start(agin[0], h0p[:])
                    nc.sync.dma_start(agin[1], h1p[:])
                    agout = rdram.tile([n_cores * 2 * 128, B], F32, tag="agout",
                                       addr_space="Shared")
                    nc.gpsimd.collective_compute(
                        "AllGather", ALU.bypass, replica_groups=rg,
                        ins=[agin.opt()], outs=[agout.opt()])

            # ================= Phase 2a: a = LN(LeakyReLU(h1 @ W1.T + b1)) ====
            with tc.tile_pool(name="aw", bufs=1) as aw, \
                 tc.tile_pool(name="ah", bufs=2) as ah, \
                 tc.tile_pool(name="atmp", bufs=2) as atmp, \
                 tc.tile_pool(name="aout", bufs=3) as aout, \
                 tc.tile_pool(name="aps", bufs=2, space="PSUM") as aps, \
                 tc.tile_pool(name="apst", bufs=2, space="PSUM") as apst:
                w1s = aw.tile([128, KH, H], F32)
                nc.sync.dma_start(w1s[:], w1T.rearrange("(k p) m -> p k m", p=128))
                b1s = aw.tile([128, KH], F32)
                nc.sync.dma_start(b1s[:], b1c[:])
                ln2ws = aw.tile([128, KH], F32)
                nc.sync.dma_start(ln2ws[:], ln2w[:])
                ln2bs = aw.tile([128, KH], F32)
                nc.sync.dma_start(ln2bs[:], ln2b[:])
                ones_col2 = aw.tile([128, 1], F32)
                nc.vector.memset(ones_col2[:], 1.0)
                ones_row2 = aw.tile([1, 128], F32)
                nc.vector.memset(ones_row2[:], 1.0)

                nbt2 = (BT + 511) // 512
                assert BT % nbt2 == 0
                btc2 = BT // nbt2
                for j in range(nbt2):
                    hk = ah.tile([128, KH, btc2], F32, tag="hk")
                    nc.sync.dma_start(
                        hk[:], h1T[:, :, j * btc2:(j + 1) * btc2].rearrange(
                            "k p n -> p k n"))
                    atiles = []
                    ps_s2 = apst.tile([1, btc2], F32, tag="pstat2")
                    ps_q2 = apst.tile([1, btc2], F32, tag="pstat2")
                    for m in range(KH):
                        ps_a = aps.tile([128, btc2], F32, tag="ps_a")
                        for kk in range(KH):
                            nc.tensor.matmul(ps_a[:],
                                             w1s[:, kk, m * 128:(m + 1) * 128],
                                             hk[:, kk, :],
                                             start=(kk == 0), stop=(kk == KH - 1))
                        rl = aout.tile([128, btc2], F32, tag="rl")
                        nc.scalar.activation(rl[:], ps_a[:], AF.Relu,
                                             bias=b1s[:, m:m + 1])
                        at = atmp.tile([128, btc2], F32, tag=f"at{m}")
                        # leaky_relu(y) = alpha*(y - relu(y)) + relu(y)
                        nc.vector.scalar_tensor_tensor(
                            at[:], ps_a[:], b1s[:, m:m + 1], rl[:],
                            ALU.add, ALU.subtract)
                        nc.vector.scalar_tensor_tensor(
                            at[:], at[:], NEG_SLOPE, rl[:], ALU.mult, ALU.add)
                        atiles.append(at)
                        nc.tensor.matmul(ps_s2[:], ones_col2[:], at[:],
                                         start=(m == 0), stop=(m == KH - 1))
                        sq2 = aout.tile([128, btc2], F32, tag="sq2")
                        nc.scalar.activation(sq2[:], at[:], AF.Square)
                        nc.tensor.matmul(ps_q2[:], ones_col2[:], sq2[:],
                                         start=(m == 0), stop=(m == KH - 1))
                    mean2 = atmp.tile([1, btc2], F32, tag="mean2")
                    nc.vector.tensor_scalar_mul(mean2[:], ps_s2[:], 1.0 / H)
                    var2 = atmp.tile([1, btc2], F32, tag="var2")
                    nc.vector.tensor_mul(var2[:], mean2[:], mean2[:])
                    nc.vector.scalar_tensor_tensor(var2[:], ps_q2[:], 1.0 / H,
                                                   var2[:], ALU.mult, ALU.subtract)
                    nc.vector.tensor_scalar_add(var2[:], var2[:], EPS)
                    nc.vector.reciprocal(var2[:], var2[:])
                    rstd2 = atmp.tile([1, btc2], F32, tag="rstd2")
                    nc.scalar.activation(rstd2[:], var2[:], AF.Sqrt)
                    ps_mb2 = apst.tile([128, btc2], F32, tag="pstat2")
                    nc.tensor.matmul(ps_mb2[:], ones_row2[:],
                                     mean2[:], start=True, stop=True)
                    ps_rb2 = apst.tile([128, btc2], F32, tag="pstat2")
                    nc.tensor.matmul(ps_rb2[:], ones_row2[:],
                                     rstd2[:], start=True, stop=True)
                    for m in range(KH):
                        at = atiles[m]
                        an = aout.tile([128, btc2], F32, tag="an")
                        nc.vector.tensor_sub(an[:], at[:], ps_mb2[:])
                        nc.vector.tensor_mul(an[:], an[:], ps_rb2[:])
                        nc.vector.tensor_scalar(an[:], an[:], ln2ws[:, m:m + 1],
                                                ln2bs[:, m:m + 1],
                                                ALU.mult, ALU.add)
                        nc.sync.dma_start(aTn[m, :, j * btc2:(j + 1) * btc2], an[:])

            # ================= Phase 2b: logits = aTn.T @ W2cT =================
            with tc.tile_pool(name="lw", bufs=1) as lw, \
                 tc.tile_pool(name="la", bufs=2) as la, \
                 tc.tile_pool(name="lo", bufs=3) as lo, \
                 tc.tile_pool(name="lps", bufs=1, space="PSUM") as lps:
                w2s = lw.tile([128, KH, VC], F32)
                nc.sync.dma_start(w2s[:], w2cT.rearrange("(k p) v -> p k v", p=128))
                nvc = 8
                vcs = VC // nvc  # 500
                mtw = min(128, BT)
                for mt in range(BT // mtw):
                    ast = la.tile([128, KH, mtw], F32, tag="ast")
                    nc.sync.dma_start(
                        ast[:], aTn[:, :, mt * mtw:(mt + 1) * mtw].rearrange(
                            "k p n -> p k n"))
                    pvs = [lps.tile([mtw, vcs], F32, tag=f"pv{v}", name=f"pv{v}")
                           for v in range(nvc)]
                    for kk in range(KH):
                        for v in range(nvc):
                            nc.tensor.matmul(pvs[v][:], ast[:, kk, :mtw],
                                             w2s[:, kk, v * vcs:(v + 1) * vcs],
                                             start=(kk == 0), stop=(kk == KH - 1))
                    for v in range(nvc):
                        ot = lo.tile([mtw, vcs], F32, tag="ot")
                        if v % 2 == 0:
                            nc.vector.tensor_copy(ot[:], pvs[v][:])
                        else:
                            nc.scalar.copy(ot[:], pvs[v][:])
                        nc.sync.dma_start(
                            out[mt * mtw:(mt + 1) * mtw, v * vcs:(v + 1) * vcs],
                            ot[:])
    return nc


# ===================== host-side prep / post =====================

def _np(x):
    return np.asarray(x)


def prep_in_maps(inputs, T=256, n_cores=NCORES):
    """inputs: dict from setup_inputs() (numpy). Returns list of in_maps."""
    ids = _np(inputs['input']).astype(np.int64)[:, :T]          # [B, T]
    embd = _np(inputs['embd']).astype(np.float32)               # [V, E]
    BT = T * B
    # xT: [KE, 128, BT] with column index t*16+b
    x = embd[ids]                                               # [B, T, E]
    xT = np.ascontiguousarray(x.transpose(2, 1, 0).reshape(E, T * B))  # [E, (t b)]
    xT = xT.reshape(KE, 128, BT)

    def gate_slice(W, c):
        # W: [3H, D] -> per-core [D, 384] with cols (r,z,n) x 128
        cols = []
        for g in range(3):
            cols.append(W[g * H + c * 128:(g * H + (c + 1) * 128), :])  # [128, D]
        Wc = np.concatenate(cols, axis=0)                        # [384, D]
        return np.ascontiguousarray(Wc.T)                        # [D, 384]

    def bias_slice(b, c, g):
        return b[g * H + c * 128:g * H + (c + 1) * 128]

    lnw = np.stack([_np(inputs['ln0_w']), _np(inputs['ln1_w'])], 0)  # [2, H]
    lnb = np.stack([_np(inputs['ln0_b']), _np(inputs['ln1_b'])], 0)
    lnw_t = np.ascontiguousarray(
        lnw.reshape(2, KH, 128).transpose(2, 0, 1)).astype(np.float32)
    lnb_t = np.ascontiguousarray(
        lnb.reshape(2, KH, 128).transpose(2, 0, 1)).astype(np.float32)
    ln2w_t = np.ascontiguousarray(
        _np(inputs['ln2_w']).reshape(KH, 128).T).astype(np.float32)
    ln2b_t = np.ascontiguousarray(
        _np(inputs['ln2_b']).reshape(KH, 128).T).astype(np.float32)
    b1_t = np.ascontiguousarray(
        _np(inputs['b1']).reshape(KH, 128).T).astype(np.float32)
    w1T = np.ascontiguousarray(_np(inputs['W1']).astype(np.float32).T)  # [H, H]
    W2 = _np(inputs['W2']).astype(np.float32)

    Wih0 = _np(inputs['Wih0']).astype(np.float32)
    Whh0 = _np(inputs['Whh0']).astype(np.float32)
    Wih1 = _np(inputs['Wih1']).astype(np.float32)
    Whh1 = _np(inputs['Whh1']).astype(np.float32)
    bih0 = _np(inputs['bih0']).astype(np.float32)
    bhh0 = _np(inputs['bhh0']).astype(np.float32)
    bih1 = _np(inputs['bih1']).astype(np.float32)
    bhh1 = _np(inputs['bhh1']).astype(np.float32)

    in_maps = []
    for c in range(n_cores):
        bih0c = np.stack([bias_slice(bih0, c, g) for g in range(3)], 1)  # [128,3]
        gb0 = np.stack([bias_slice(bhh0, c, g) for g in range(3)], 1)
        gb1 = np.stack([
            bias_slice(bih1, c, 0) + bias_slice(bhh1, c, 0),
            bias_slice(bih1, c, 1) + bias_slice(bhh1, c, 1),
            bias_slice(bih1, c, 2),
            bias_slice(bhh1, c, 2)], 1)                                  # [128,4]
        maskc = np.zeros((128, KH), np.float32)
        maskc[:, c] = 1.0
        eye16 = np.eye(16, dtype=np.float32)
        w2cT = np.ascontiguousarray(W2[c * VC:(c + 1) * VC, :].T)        # [H, VC]
        in_maps.append({
            'xT': xT, 'wih0': gate_slice(Wih0, c), 'whh0': gate_slice(Whh0, c),
            'wih1': gate_slice(Wih1, c), 'whh1': gate_slice(Whh1, c),
            'bih0c': np.ascontiguousarray(bih0c),
            'gb0': np.ascontiguousarray(gb0), 'gb1': np.ascontiguousarray(gb1),
            'lnw': lnw_t, 'lnb': lnb_t, 'maskc': maskc, 'eye16': eye16,
            'ln2w': ln2w_t, 'ln2b': ln2b_t, 'b1c': b1_t,
            'w1T': w1T, 'w2cT': w2cT,
        })
    return in_maps


def postprocess(results, inputs, T=256):
    """results: list of per-core {'out': [BT, VC]}. Returns [B, T, V]."""
    b2 = _np(inputs['b2']).astype(np.float32)
    full = np.concatenate([r['out'] for r in results], axis=1)  # [BT, V]
    full = full.reshape(T, B, V).transpose(1, 0, 2)             # [B, T, V]
    return full + b2


# ===================== numpy mirror (for sim testing) =====================

def numpy_reference(inputs, T=256):
    ids = _np(inputs['input']).astype(np.int64)[:, :T]
    embd = _np(inputs['embd'])
    x = embd[ids].astype(np.float32)        # [B, T, E]
    h0 = np.zeros((B, H), np.float32)
    h1 = np.zeros((B, H), np.float32)

    def ln(v, w, bb):
        m = v.mean(-1, keepdims=True)
        var = v.var(-1, keepdims=True)
        return (v - m) / np.sqrt(var + EPS) * w + bb

    def gru(xx, hh, Wih, Whh, bih, bhh):
        gi = xx @ _np(Wih).T + _np(bih)
        gh = hh @ _np(Whh).T + _np(bhh)
        ir, iz, inn = np.split(gi, 3, -1)
        hr, hz, hn_ = np.split(gh, 3, -1)
        r = 1 / (1 + np.exp(-(ir + hr)))
        z = 1 / (1 + np.exp(-(iz + hz)))
        n = np.tanh(inn + r * hn_)
        return (1 - z) * n + z * hh

    outs = []
    for t in range(T):
        h0 = ln(gru(x[:, t], h0, inputs['Wih0'], inputs['Whh0'],
                    inputs['bih0'], inputs['bhh0']),
                _np(inputs['ln0_w']), _np(inputs['ln0_b']))
        h1 = ln(gru(h0, h1, inputs['Wih1'], inputs['Whh1'],
                    inputs['bih1'], inputs['bhh1']),
                _np(inputs['ln1_w']), _np(inputs['ln1_b']))
        a = h1 @ _np(inputs['W1']).T + _np(inputs['b1'])
        a = np.where(a > 0, a, NEG_SLOPE * a)
        a = ln(a, _np(inputs['ln2_w']), _np(inputs['ln2_b']))
        outs.append(a @ _np(inputs['W2']).T + _np(inputs['b2']))
    return np.stack(outs, 1)  # [B, T, V]


# ===================== NEFF disk cache =====================

def _install_neff_cache():
    import hashlib, os, shutil
    import concourse.bass2jax as b2j
    from concourse.bass_utils import compile_bir_kernel as _real
    if getattr(b2j, "_ant_neff_cache_installed", False):
        return
    cache_dir = os.path.expanduser("~/.cache/bass_neff_cache")
    os.makedirs(cache_dir, exist_ok=True)

    def cached(bir_json, tmpdir, neff_name="file.neff"):
        key = hashlib.sha256(bir_json).hexdigest()
        p = os.path.join(cache_dir, key + ".neff")
        out = os.path.join(tmpdir, neff_name)
        if os.path.exists(p):
            shutil.copyfile(p, out)
            return out
        r = _real(bir_json, tmpdir, neff_name)
        try:
            shutil.copyfile(r, p)
        except OSError:
            pass
        return r

    b2j.compile_bir_kernel = cached
    b2j._ant_neff_cache_installed = True


# ===================== NTFF profile shim (for traced runs) ==================

def _install_axon_prof():
    import types, ctypes, contextlib
    try:
        from antenv import axon_hooks  # noqa: F401
        return
    except ImportError:
        pass
    so_path = "/opt/axon/libaxon_pjrt.so"
    try:
        lib = ctypes.CDLL(so_path)
    except OSError:
        return
    hook = None
    if hasattr(lib, "axon_start_nrt_profile"):
        lib.axon_start_nrt_profile.argtypes = [
            ctypes.POINTER(ctypes.c_int64), ctypes.c_size_t]
        lib.axon_start_nrt_profile.restype = ctypes.c_int64
        lib.axon_stop_nrt_profile.argtypes = [ctypes.c_char_p]
        lib.axon_stop_nrt_profile.restype = ctypes.c_int64

        @contextlib.contextmanager
        def hook(output_dir, device_ids):
            import jax
            jax.devices()
            if device_ids:
                ids = (ctypes.c_int64 * len(device_ids))(*device_ids)
                rc = lib.axon_start_nrt_profile(ids, len(device_ids))
            else:
                rc = lib.axon_start_nrt_profile(None, 0)
            if rc != 0:
                raise RuntimeError(f"axon_start_nrt_profile rc={rc}")
            try:
                yield
            finally:
                lib.axon_stop_nrt_profile(str(output_dir).encode())

    mod = types.ModuleType("antenv.axon_hooks")
    _h = [hook]
    mod.set_axon_ntff_profile_hook = lambda h: _h.__setitem__(0, h)
    mod.get_axon_ntff_profile_hook = lambda: _h[0]
    _sys.modules["antenv.axon_hooks"] = mod
    import antenv
    antenv.axon_hooks = mod


# ===================== entry point =====================

_NC = None


def _get_nc():
    global _NC
    if _NC is None:
        _install_neff_cache()
        nc = build_nc(T=256)
        nc.compile()
        _NC = nc
    return _NC


def kernel(**inputs):
    import numpy as np
    from concourse import bass_utils
    nc = _get_nc()
    in_maps = prep_in_maps(inputs, T=256)
    res = bass_utils.run_bass_kernel_spmd(
        nc, in_maps, core_ids=list(range(NCORES)))
    return postprocess(res.results, inputs, T=256)


def kernel_traced(**inputs):
    """Like kernel() but also returns neuron-profile exec_time_ns."""
    from concourse import bass_utils
    _install_axon_prof()
    nc = _get_nc()
    in_maps = prep_in_maps(inputs, T=256)
    res = bass_utils.run_bass_kernel_spmd(
        nc, in_maps, core_ids=list(range(NCORES)), trace=True)
    return postprocess(res.results, inputs, T=256), res.exec_time_ns

